# revision 16
# baseline (speedup 1.0000x reference)
"""CoAttention kernel for 8 TRN2 NeuronCores (Bass/Tile, SPMD).

Problem: B=4 batches x 2 attention directions = 8 independent co-attention
computations -> one per core.  Per core (batch b, direction d):
    Q = wq @ qf + bq        [256, 2304]     (qf = query-side features)
    K = wr @ rf + br        [256, 2304]     (rf = reference-side features)
    S^T = K^T Q             [2304, 2304]    (computed in m-strips of 128)
    attnT = exp(S^T - 40)   (bf16, unnormalized; softmax denom applied at end)
    sums[q] = sum_m attnT[m, q]             (ones-matmul over partitions)
    out = (rf @ attnT) * (1/sums)           [2048, 2304]
Host assembles: left_att = concat(left, out[b,dir=0]), right_att likewise.

Precision: score path in float32r (RNE-11-mantissa, full PE rate at free>=256,
host pre-rounds inputs), attn@V in bf16.  No row-max subtraction: scores are
|S| <~ 80, exp(S-40) stays in fp32/bf16 range; normalization is exact math.

Walrus in this toolchain allows ONE sync-wait per instruction; SafeTileContext
splits multi-wait instructions into standalone wait ops, and splits the
end-of-kernel drain the same way.
"""
import numpy as np
import ml_dtypes

import concourse.bass as bass
import concourse.mybir as mybir
import concourse.tile as tile
from concourse.vector_clock import ScopedClock
from concourse.bass_utils import run_bass_kernel_spmd

B = 4
C = 2048
HW = 48 * 48          # 2304
D = 256
NCORES = 8

CB = C // 128         # 16 c-blocks
DB = D // 128         # 2 d-blocks
MS = HW // 128        # 18 m-strips
# phase-1 n chunks: 512-wide (max f32r moving free dim) + 256 tail;
# each chunk's psum fits one 2KB PSUM bank
P1CHUNKS = [(0, 512), (512, 512), (1024, 512), (1536, 512), (2048, 256)]
NQT = 3               # phase-2 q thirds
QT = HW // NQT        # 768
# sub-chunks within a q-third: matmul outputs must not cross a 2KB PSUM
# bank boundary, so split 768 = 512 + 256 (both >=256 keeps f32r full rate)
SUBS = [(0, 512), (512, 256)]

F32 = mybir.dt.float32
F32R = mybir.dt.float32r
BF16 = mybir.dt.bfloat16

# module-level knobs / results (used by test.py)
TRACE = False
LAST_RESULT = None


class SafeTileContext(tile.TileContext):
    """This walrus build allows at most ONE sync wait per instruction.
    Hoist extra waits onto standalone EventSemaphore (wait-only) ops placed
    immediately before, on the same engine queue; same for the final drain."""
    MAX_WAITS = 1

    def _lower_ordered_insts(self, ordered):
        for bname, insts in ordered.items():
            new_list = []
            for inst in insts:
                si = inst.sync_info
                if si is not None and len(si.on_wait) > self.MAX_WAITS:
                    waits = list(si.on_wait)
                    movable = [w for w in waits if w.wait_reg is None]
                    fixed = [w for w in waits if w.wait_reg is not None]
                    keep = fixed + movable[-1:] if movable else fixed
                    hoist = movable[:-1] if movable else []
                    for w in hoist:
                        wi = mybir.InstEventSemaphore(
                            name=self.nc.get_next_instruction_name(),
                            ins=[], outs=[])
                        wi.engine = inst.engine
                        wi.sync_info = mybir.SyncInfo(on_wait=[w], on_update=[])
                        new_list.append(wi)
                    inst.sync_info = mybir.SyncInfo(
                        on_wait=keep, on_update=list(si.on_update))
                new_list.append(inst)
            insts[:] = new_list
        super()._lower_ordered_insts(ordered)

    def _drain_and_barrier(self, tick_clock, wait_clock):
        drain_inst = self.nc.sync.drain()
        wait_clock.add_sem_waits(
            drain_inst.ins, ScopedClock({None: tick_clock.global_clock}))
        si = drain_inst.ins.sync_info
        waits = list(si.on_wait) if si is not None else []
        ups = list(si.on_update) if si is not None else []
        if len(waits) > self.MAX_WAITS:
            drain_inst.ins.sync_info = mybir.SyncInfo(
                on_wait=waits[: self.MAX_WAITS], on_update=ups)
            rest = waits[self.MAX_WAITS:]
            for i in range(0, len(rest), self.MAX_WAITS):
                extra = self.nc.sync.drain()
                extra.ins.sync_info = mybir.SyncInfo(
                    on_wait=rest[i : i + self.MAX_WAITS], on_update=[])
        self.nc.all_engine_barrier()
        assert self.sems is not None
        popped = self.nc._tile_sem_poison_stack.pop()
        assert popped is self._sem_poison
        self.nc.clear_and_free_semaphores(list(self.sems.allocated().values()))
        self.nc.all_engine_barrier()


def build_kernel(dumps=False):
    nc = bass.Bass("TRN2", target_bir_lowering=False, debug=False)

    qf = nc.dram_tensor("qf", [C, HW], F32R, kind="ExternalInput")
    rf = nc.dram_tensor("rf", [C, HW], F32R, kind="ExternalInput")
    vtb = nc.dram_tensor("vtb", [HW, C], BF16, kind="ExternalInput")
    wqt = nc.dram_tensor("wqt", [C, D], F32R, kind="ExternalInput")
    wrt = nc.dram_tensor("wrt", [C, D], F32R, kind="ExternalInput")
    bq = nc.dram_tensor("bq", [128, DB], F32, kind="ExternalInput")
    br = nc.dram_tensor("br", [128, DB], F32, kind="ExternalInput")
    out = nc.dram_tensor("out", [C, HW], F32, kind="ExternalOutput")
    if dumps:
        qdump = nc.dram_tensor("qdump", [128, DB, HW], F32, kind="ExternalOutput")
        kdump = nc.dram_tensor("kdump", [128, DB, HW], F32, kind="ExternalOutput")
        sdump = nc.dram_tensor("sdump", [NQT, QT], F32, kind="ExternalOutput")
        adump = nc.dram_tensor("adump", [128, MS, QT], BF16, kind="ExternalOutput")

    with SafeTileContext(nc) as tc:
        with tc.tile_pool(name="persist", bufs=1) as persist, \
             tc.tile_pool(name="dsc", bufs=1, space="DRAM") as dram_scratch:
            # ---- persistent tiles ----
            q_sb = persist.tile([128, DB, HW], F32R)    # Q  [d, n]
            k_sb = persist.tile([128, DB, HW], F32R)    # K  [d, n]
            vt = persist.tile([128, MS, C], BF16)       # V^T [m, c]
            bq_t = persist.tile([128, DB], F32)
            br_t = persist.tile([128, DB], F32)
            nc.sync.dma_start(out=bq_t, in_=bq.ap())
            nc.sync.dma_start(out=br_t, in_=br.ap())
            nbias = persist.tile([128, 1], F32)
            nc.vector.memset(nbias, -40.0)
            ones = persist.tile([128, 1], BF16)
            nc.vector.memset(ones, 1.0)

            # ================= phase 1: projections + V^T =================
            with tc.tile_pool(name="wpool", bufs=1) as wpool, \
                 tc.tile_pool(name="xstream", bufs=4) as xstream, \
                 tc.tile_pool(name="p1ps", bufs=1, space="PSUM") as p1ps:
                wq_sb = wpool.tile([128, CB, D], F32R)
                wr_sb = wpool.tile([128, CB, D], F32R)
                wqr = wqt.ap().rearrange("(k p) d -> p k d", p=128)
                wrr = wrt.ap().rearrange("(k p) d -> p k d", p=128)
                for h in range(4):
                    cs = slice(h * 4, (h + 1) * 4)
                    nc.sync.dma_start(out=wq_sb[:, cs, :], in_=wqr[:, cs, :])
                    nc.sync.dma_start(out=wr_sb[:, cs, :], in_=wrr[:, cs, :])

                for ch, (coff, csz) in enumerate(P1CHUNKS):
                    qps = [p1ps.tile([128, 512], F32, tag=f"qps{d}",
                                     name=f"qps{d}_{ch}")
                           for d in range(DB)]
                    kps = [p1ps.tile([128, 512], F32, tag=f"kps{d}",
                                     name=f"kps{d}_{ch}")
                           for d in range(DB)]
                    qfr = qf.ap().rearrange("(k p) n -> p k n", p=128)
                    rfr = rf.ap().rearrange("(k p) n -> p k n", p=128)
                    for quad in range(CB // 4):
                        xq = xstream.tile([128, 4, 512], F32R, tag="xq",
                                          name=f"xq_{ch}_{quad}")
                        nc.sync.dma_start(
                            out=xq[:, :, :csz],
                            in_=qfr[:, quad * 4:(quad + 1) * 4,
                                    coff:coff + csz])
                        xr = xstream.tile([128, 4, 512], F32R, tag="xr",
                                          name=f"xr_{ch}_{quad}")
                        nc.sync.dma_start(
                            out=xr[:, :, :csz],
                            in_=rfr[:, quad * 4:(quad + 1) * 4,
                                    coff:coff + csz])
                        for i in range(4):
                            c = quad * 4 + i
                            for d in range(DB):
                                nc.tensor.matmul(
                                    qps[d][:, :csz],
                                    wq_sb[:, c, d * 128:(d + 1) * 128],
                                    xq[:, i, :csz],
                                    start=(c == 0), stop=(c == CB - 1))
                                nc.tensor.matmul(
                                    kps[d][:, :csz],
                                    wr_sb[:, c, d * 128:(d + 1) * 128],
                                    xr[:, i, :csz],
                                    start=(c == 0), stop=(c == CB - 1))
                    for d in range(DB):
                        nc.vector.tensor_scalar_add(
                            q_sb[:, d, coff:coff + csz],
                            qps[d][:, :csz], bq_t[:, d:d + 1])
                        nc.vector.tensor_scalar_add(
                            k_sb[:, d, coff:coff + csz],
                            kps[d][:, :csz], br_t[:, d:d + 1])

            # V^T (host-pretransposed bf16): plain strided loads, emitted
            # after phase 1 so they overlap the first S^T strips (vt is first
            # read in av_phase)
            vtr = vtb.ap().rearrange("(s p) c -> p s c", p=128)
            for h in range(6):
                nc.sync.dma_start(out=vt[:, 3 * h:3 * (h + 1), :],
                                  in_=vtr[:, 3 * h:3 * (h + 1), :])

            if dumps:
                nc.sync.dma_start(out=qdump.ap(), in_=q_sb.bitcast(F32))
                nc.sync.dma_start(out=kdump.ap(), in_=k_sb.bitcast(F32))

            # ================= phase 2: S^T, softmax, attn@V ==============
            with tc.tile_pool(name="attn", bufs=2) as attnp, \
                 tc.tile_pool(name="small", bufs=2) as small, \
                 tc.tile_pool(name="ostage", bufs=3) as ostage, \
                 tc.tile_pool(name="sps", bufs=2, space="PSUM") as spsp, \
                 tc.tile_pool(name="sums", bufs=1, space="PSUM") as sumsp, \
                 tc.tile_pool(name="ops", bufs=2, space="PSUM") as opsp:
                attn_ts = {}
                bcs = {}

                def st_phase(qt):
                    attn_t = attnp.tile([128, MS, QT], BF16, tag="attnT",
                                        name=f"attnT_{qt}")
                    attn_ts[qt] = attn_t
                    sums_ps = [sumsp.tile([1, sz], F32, tag=f"sums{s}",
                                          name=f"sums{s}_{qt}")
                               for s, (off, sz) in enumerate(SUBS)]
                    for m in range(MS):
                        sps = spsp.tile([128, QT], F32, tag="sps")
                        for off, sz in SUBS:
                            for d in range(DB):
                                nc.tensor.matmul(
                                    sps[:, off:off + sz],
                                    k_sb[:, d, m * 128:(m + 1) * 128],
                                    q_sb[:, d, qt * QT + off:
                                         qt * QT + off + sz],
                                    start=(d == 0), stop=(d == DB - 1))
                        nc.scalar.activation(
                            attn_t[:, m, :], sps,
                            mybir.ActivationFunctionType.Exp,
                            bias=nbias, scale=1.0)
                        for s, (off, sz) in enumerate(SUBS):
                            nc.tensor.matmul(
                                sums_ps[s], ones,
                                attn_t[:, m, off:off + sz],
                                start=(m == 0), stop=(m == MS - 1))
                    # inv-sums -> broadcast tile [128, QT]
                    sums_sb = small.tile([1, QT], F32, tag="sums_sb",
                                         name=f"sums_sb_{qt}")
                    for s, (off, sz) in enumerate(SUBS):
                        nc.scalar.copy(
                            sums_sb[:, off:off + sz], sums_ps[s])
                    if dumps:
                        nc.sync.dma_start(out=sdump.ap()[qt:qt+1, :], in_=sums_sb)
                    invs = small.tile([1, QT], F32, tag="invs",
                                      name=f"invs_{qt}")
                    nc.vector.reciprocal(invs, sums_sb)
                    invs_dram = dram_scratch.tile([1, QT], F32, tag="invd",
                                                  name=f"invd_{qt}", bufs=2)
                    nc.sync.dma_start(out=invs_dram, in_=invs)
                    bc = small.tile([128, QT], F32, tag="bc", name=f"bc_{qt}")
                    bcs[qt] = bc
                    nc.sync.dma_start(out=bc, in_=invs_dram.partition_broadcast(128))
                    if dumps and qt == 0:
                        nc.sync.dma_start(out=adump.ap(), in_=attn_t)

                def av_phase(qt):
                    attn_t = attn_ts.pop(qt)
                    bc = bcs.pop(qt)
                    for cb in range(CB):
                        o_sb = ostage.tile([128, QT], F32, tag="osb",
                                           name=f"osb_{qt}_{cb}")
                        for off, sz in SUBS:
                            ops = opsp.tile([128, 512], F32, tag="ops")
                            nc.tensor.matmul(
                                ops[:, :sz], vt[:, 0, cb * 128:(cb + 1) * 128],
                                attn_t[:, 0, off:off + sz],
                                start=True, stop=False)
                            for m in range(1, MS):
                                nc.tensor.matmul(
                                    ops[:, :sz], vt[:, m, cb * 128:(cb + 1) * 128],
                                    attn_t[:, m, off:off + sz],
                                    start=False, stop=(m == MS - 1))
                            nc.vector.scalar_tensor_tensor(
                                o_sb[:, off:off + sz], ops[:, :sz], 0.0,
                                bc[:, off:off + sz],
                                op0=mybir.AluOpType.add,
                                op1=mybir.AluOpType.mult)
                        nc.gpsimd.dma_start(
                            out=out.ap()[cb * 128:(cb + 1) * 128,
                                         qt * QT:(qt + 1) * QT],
                            in_=o_sb)

                for qt in range(NQT):
                    st_phase(qt)
                    if qt >= 1:
                        av_phase(qt - 1)
                av_phase(NQT - 1)
    return nc


def _round_f32r(x):
    """Round-to-nearest-even to 11 mantissa bits (float32r semantics)."""
    u = np.ascontiguousarray(x, dtype=np.float32).view(np.uint32)
    rb = np.uint32(1 << 11)
    mask = np.uint32(0xFFFFF000)
    return ((u + rb) & mask).view(np.float32)


def kernel(left_features, right_features, wq, bq, wr, br):
    global LAST_RESULT
    left = np.asarray(left_features, dtype=np.float32)
    right = np.asarray(right_features, dtype=np.float32)
    wq = np.asarray(wq, dtype=np.float32)
    wr = np.asarray(wr, dtype=np.float32)
    bq = np.asarray(bq, dtype=np.float32)
    br = np.asarray(br, dtype=np.float32)

    lf = left.reshape(B, C, HW)
    rg = right.reshape(B, C, HW)
    lf_r = _round_f32r(lf)
    rg_r = _round_f32r(rg)
    wqt = _round_f32r(np.ascontiguousarray(wq.T))      # [C, D]
    wrt = _round_f32r(np.ascontiguousarray(wr.T))
    bq_t = np.ascontiguousarray(bq.reshape(DB, 128).T)  # [128, DB]
    br_t = np.ascontiguousarray(br.reshape(DB, 128).T)

    nc = build_kernel()
    in_maps = []
    for core in range(NCORES):
        b, d = core // 2, core % 2
        qf_c = lf_r[b] if d == 0 else rg_r[b]
        rf_c = rg_r[b] if d == 0 else lf_r[b]
        in_maps.append({
            "qf": np.ascontiguousarray(qf_c),
            "rf": np.ascontiguousarray(rf_c),
            "vtb": np.ascontiguousarray(rf_c.T.astype(ml_dtypes.bfloat16)),
            "wqt": wqt, "wrt": wrt, "bq": bq_t, "br": br_t,
        })
    res = run_bass_kernel_spmd(nc, in_maps, core_ids=list(range(NCORES)),
                               trace=TRACE)
    LAST_RESULT = res

    weighted = np.stack([res.results[core]["out"] for core in range(NCORES)])
    weighted = weighted.reshape(B, 2, C, 48, 48)
    left_att = np.concatenate([left, weighted[:, 0]], axis=1)
    right_att = np.concatenate([right, weighted[:, 1]], axis=1)
    return (left_att, right_att)


# revision 17
# speedup vs baseline: 1.1545x; 1.1545x over previous
"""CoAttention kernel for 8 TRN2 NeuronCores (Bass/Tile, SPMD).

Problem: B=4 batches x 2 attention directions = 8 independent co-attention
computations -> one per core.  Per core (batch b, direction d):
    Q = wq @ qf + bq        [256, 2304]     (qf = query-side features)
    K = wr @ rf + br        [256, 2304]     (rf = reference-side features)
    S^T = K^T Q             [2304, 2304]    (computed in m-strips of 128)
    attnT = exp(S^T - 40)   (bf16, unnormalized; softmax denom applied at end)
    sums[q] = sum_m attnT[m, q]             (ones-matmul over partitions)
    out = (rf @ attnT) * (1/sums)           [2048, 2304]
Host assembles: left_att = concat(left, out[b,dir=0]), right_att likewise.

Precision: score path in float32r (RNE-11-mantissa, full PE rate at free>=256,
host pre-rounds inputs), attn@V in bf16.  No row-max subtraction: scores are
|S| <~ 80, exp(S-40) stays in fp32/bf16 range; normalization is exact math.

Walrus in this toolchain allows ONE sync-wait per instruction; SafeTileContext
splits multi-wait instructions into standalone wait ops, and splits the
end-of-kernel drain the same way.
"""
import numpy as np
import ml_dtypes

import concourse.bass as bass
import concourse.mybir as mybir
import concourse.tile as tile
from concourse.vector_clock import ScopedClock
from concourse.bass_utils import run_bass_kernel_spmd

B = 4
C = 2048
HW = 48 * 48          # 2304
D = 256
NCORES = 8

CB = C // 128         # 16 c-blocks
DB = D // 128         # 2 d-blocks
MS = HW // 128        # 18 m-strips
# phase-1 n chunks: 512-wide (max f32r moving free dim) + 256 tail;
# each chunk's psum fits one 2KB PSUM bank
P1CHUNKS = [(0, 512), (512, 512), (1024, 512), (1536, 512), (2048, 256)]
NQT = 3               # phase-2 q thirds
QT = HW // NQT        # 768
# sub-chunks within a q-third: matmul outputs must not cross a 2KB PSUM
# bank boundary, so split 768 = 512 + 256 (both >=256 keeps f32r full rate)
SUBS = [(0, 512), (512, 256)]

F32 = mybir.dt.float32
F32R = mybir.dt.float32r
BF16 = mybir.dt.bfloat16

# module-level knobs / results (used by test.py)
TRACE = False
LAST_RESULT = None


class SafeTileContext(tile.TileContext):
    """This walrus build allows at most ONE sync wait per instruction.
    Hoist extra waits onto standalone EventSemaphore (wait-only) ops placed
    immediately before, on the same engine queue; same for the final drain."""
    MAX_WAITS = 1

    def _lower_ordered_insts(self, ordered):
        for bname, insts in ordered.items():
            new_list = []
            for inst in insts:
                si = inst.sync_info
                if si is not None and len(si.on_wait) > self.MAX_WAITS:
                    waits = list(si.on_wait)
                    movable = [w for w in waits if w.wait_reg is None]
                    fixed = [w for w in waits if w.wait_reg is not None]
                    keep = fixed + movable[-1:] if movable else fixed
                    hoist = movable[:-1] if movable else []
                    for w in hoist:
                        wi = mybir.InstEventSemaphore(
                            name=self.nc.get_next_instruction_name(),
                            ins=[], outs=[])
                        wi.engine = inst.engine
                        wi.sync_info = mybir.SyncInfo(on_wait=[w], on_update=[])
                        new_list.append(wi)
                    inst.sync_info = mybir.SyncInfo(
                        on_wait=keep, on_update=list(si.on_update))
                new_list.append(inst)
            insts[:] = new_list
        super()._lower_ordered_insts(ordered)

    def _drain_and_barrier(self, tick_clock, wait_clock):
        drain_inst = self.nc.sync.drain()
        wait_clock.add_sem_waits(
            drain_inst.ins, ScopedClock({None: tick_clock.global_clock}))
        si = drain_inst.ins.sync_info
        waits = list(si.on_wait) if si is not None else []
        ups = list(si.on_update) if si is not None else []
        if len(waits) > self.MAX_WAITS:
            drain_inst.ins.sync_info = mybir.SyncInfo(
                on_wait=waits[: self.MAX_WAITS], on_update=ups)
            rest = waits[self.MAX_WAITS:]
            for i in range(0, len(rest), self.MAX_WAITS):
                extra = self.nc.sync.drain()
                extra.ins.sync_info = mybir.SyncInfo(
                    on_wait=rest[i : i + self.MAX_WAITS], on_update=[])
        self.nc.all_engine_barrier()
        assert self.sems is not None
        popped = self.nc._tile_sem_poison_stack.pop()
        assert popped is self._sem_poison
        self.nc.clear_and_free_semaphores(list(self.sems.allocated().values()))
        self.nc.all_engine_barrier()


def build_kernel(dumps=False):
    nc = bass.Bass("TRN2", target_bir_lowering=False, debug=False)

    qf = nc.dram_tensor("qf", [C, HW], F32R, kind="ExternalInput")
    rf = nc.dram_tensor("rf", [C, HW], F32R, kind="ExternalInput")
    vtb = nc.dram_tensor("vtb", [HW, C], BF16, kind="ExternalInput")
    wqt = nc.dram_tensor("wqt", [C, D], F32R, kind="ExternalInput")
    wrt = nc.dram_tensor("wrt", [C, D], F32R, kind="ExternalInput")
    bq = nc.dram_tensor("bq", [128, DB], F32, kind="ExternalInput")
    br = nc.dram_tensor("br", [128, DB], F32, kind="ExternalInput")
    out = nc.dram_tensor("out", [C, HW], F32, kind="ExternalOutput")
    if dumps:
        qdump = nc.dram_tensor("qdump", [128, DB, HW], F32, kind="ExternalOutput")
        kdump = nc.dram_tensor("kdump", [128, DB, HW], F32, kind="ExternalOutput")
        sdump = nc.dram_tensor("sdump", [NQT, QT], F32, kind="ExternalOutput")
        adump = nc.dram_tensor("adump", [128, MS, QT], BF16, kind="ExternalOutput")

    with SafeTileContext(nc) as tc:
        with tc.tile_pool(name="persist", bufs=1) as persist, \
             tc.tile_pool(name="dsc", bufs=1, space="DRAM") as dram_scratch:
            # ---- persistent tiles ----
            q_sb = persist.tile([128, DB, HW], F32R)    # Q  [d, n]
            k_sb = persist.tile([128, DB, HW], F32R)    # K  [d, n]
            vt = persist.tile([128, MS, C], BF16)       # V^T [m, c]
            bq_t = persist.tile([128, DB], F32)
            br_t = persist.tile([128, DB], F32)
            nc.sync.dma_start(out=bq_t, in_=bq.ap())
            nc.sync.dma_start(out=br_t, in_=br.ap())
            nbias = persist.tile([128, 1], F32)
            nc.vector.memset(nbias, -40.0)
            ones = persist.tile([128, 1], BF16)
            nc.vector.memset(ones, 1.0)

            # ================= phase 1: projections + V^T =================
            with tc.tile_pool(name="wpool", bufs=1) as wpool, \
                 tc.tile_pool(name="xstream", bufs=4) as xstream, \
                 tc.tile_pool(name="p1ps", bufs=1, space="PSUM") as p1ps:
                wq_sb = wpool.tile([128, CB, D], F32R)
                wr_sb = wpool.tile([128, CB, D], F32R)
                wqr = wqt.ap().rearrange("(k p) d -> p k d", p=128)
                wrr = wrt.ap().rearrange("(k p) d -> p k d", p=128)
                for h in range(2):
                    cs = slice(h * 8, (h + 1) * 8)
                    nc.sync.dma_start(out=wq_sb[:, cs, :], in_=wqr[:, cs, :])
                    nc.sync.dma_start(out=wr_sb[:, cs, :], in_=wrr[:, cs, :])

                for ch, (coff, csz) in enumerate(P1CHUNKS):
                    qps = [p1ps.tile([128, 512], F32, tag=f"qps{d}",
                                     name=f"qps{d}_{ch}")
                           for d in range(DB)]
                    kps = [p1ps.tile([128, 512], F32, tag=f"kps{d}",
                                     name=f"kps{d}_{ch}")
                           for d in range(DB)]
                    qfr = qf.ap().rearrange("(k p) n -> p k n", p=128)
                    rfr = rf.ap().rearrange("(k p) n -> p k n", p=128)
                    for quad in range(CB // 4):
                        xq = xstream.tile([128, 4, 512], F32R, tag="xq",
                                          name=f"xq_{ch}_{quad}")
                        nc.sync.dma_start(
                            out=xq[:, :, :csz],
                            in_=qfr[:, quad * 4:(quad + 1) * 4,
                                    coff:coff + csz])
                        xr = xstream.tile([128, 4, 512], F32R, tag="xr",
                                          name=f"xr_{ch}_{quad}")
                        nc.sync.dma_start(
                            out=xr[:, :, :csz],
                            in_=rfr[:, quad * 4:(quad + 1) * 4,
                                    coff:coff + csz])
                        for i in range(4):
                            c = quad * 4 + i
                            for d in range(DB):
                                nc.tensor.matmul(
                                    qps[d][:, :csz],
                                    wq_sb[:, c, d * 128:(d + 1) * 128],
                                    xq[:, i, :csz],
                                    start=(c == 0), stop=(c == CB - 1))
                                nc.tensor.matmul(
                                    kps[d][:, :csz],
                                    wr_sb[:, c, d * 128:(d + 1) * 128],
                                    xr[:, i, :csz],
                                    start=(c == 0), stop=(c == CB - 1))
                    for d in range(DB):
                        nc.vector.tensor_scalar_add(
                            q_sb[:, d, coff:coff + csz],
                            qps[d][:, :csz], bq_t[:, d:d + 1])
                        nc.vector.tensor_scalar_add(
                            k_sb[:, d, coff:coff + csz],
                            kps[d][:, :csz], br_t[:, d:d + 1])

            # V^T (host-pretransposed bf16): plain strided loads, emitted
            # after phase 1 so they overlap the first S^T strips (vt is first
            # read in av_phase)
            vtr = vtb.ap().rearrange("(s p) c -> p s c", p=128)
            for h in range(6):
                nc.sync.dma_start(out=vt[:, 3 * h:3 * (h + 1), :],
                                  in_=vtr[:, 3 * h:3 * (h + 1), :])

            if dumps:
                nc.sync.dma_start(out=qdump.ap(), in_=q_sb.bitcast(F32))
                nc.sync.dma_start(out=kdump.ap(), in_=k_sb.bitcast(F32))

            # ================= phase 2: S^T, softmax, attn@V ==============
            with tc.tile_pool(name="attn", bufs=2) as attnp, \
                 tc.tile_pool(name="small", bufs=2) as small, \
                 tc.tile_pool(name="ostage", bufs=3) as ostage, \
                 tc.tile_pool(name="sps", bufs=2, space="PSUM") as spsp, \
                 tc.tile_pool(name="sums", bufs=1, space="PSUM") as sumsp, \
                 tc.tile_pool(name="ops", bufs=2, space="PSUM") as opsp:
                attn_ts = {}
                bcs = {}

                def st_phase(qt):
                    attn_t = attnp.tile([128, MS, QT], BF16, tag="attnT",
                                        name=f"attnT_{qt}")
                    attn_ts[qt] = attn_t
                    sums_ps = [sumsp.tile([1, sz], F32, tag=f"sums{s}",
                                          name=f"sums{s}_{qt}")
                               for s, (off, sz) in enumerate(SUBS)]
                    for m in range(MS):
                        sps = spsp.tile([128, QT], F32, tag="sps")
                        for off, sz in SUBS:
                            for d in range(DB):
                                nc.tensor.matmul(
                                    sps[:, off:off + sz],
                                    k_sb[:, d, m * 128:(m + 1) * 128],
                                    q_sb[:, d, qt * QT + off:
                                         qt * QT + off + sz],
                                    start=(d == 0), stop=(d == DB - 1))
                        nc.scalar.activation(
                            attn_t[:, m, :], sps,
                            mybir.ActivationFunctionType.Exp,
                            bias=nbias, scale=1.0)
                        for s, (off, sz) in enumerate(SUBS):
                            nc.tensor.matmul(
                                sums_ps[s], ones,
                                attn_t[:, m, off:off + sz],
                                start=(m == 0), stop=(m == MS - 1))
                    # inv-sums -> broadcast tile [128, QT]
                    sums_sb = small.tile([1, QT], F32, tag="sums_sb",
                                         name=f"sums_sb_{qt}")
                    for s, (off, sz) in enumerate(SUBS):
                        nc.scalar.copy(
                            sums_sb[:, off:off + sz], sums_ps[s])
                    if dumps:
                        nc.sync.dma_start(out=sdump.ap()[qt:qt+1, :], in_=sums_sb)
                    invs = small.tile([1, QT], F32, tag="invs",
                                      name=f"invs_{qt}")
                    nc.vector.reciprocal(invs, sums_sb)
                    invs_dram = dram_scratch.tile([1, QT], F32, tag="invd",
                                                  name=f"invd_{qt}", bufs=2)
                    nc.sync.dma_start(out=invs_dram, in_=invs)
                    bc = small.tile([128, QT], F32, tag="bc", name=f"bc_{qt}")
                    bcs[qt] = bc
                    nc.sync.dma_start(out=bc, in_=invs_dram.partition_broadcast(128))
                    if dumps and qt == 0:
                        nc.sync.dma_start(out=adump.ap(), in_=attn_t)

                def av_phase(qt):
                    attn_t = attn_ts.pop(qt)
                    bc = bcs.pop(qt)
                    for cb in range(CB):
                        o_sb = ostage.tile([128, QT], F32, tag="osb",
                                           name=f"osb_{qt}_{cb}")
                        for off, sz in SUBS:
                            ops = opsp.tile([128, 512], F32, tag="ops")
                            nc.tensor.matmul(
                                ops[:, :sz], vt[:, 0, cb * 128:(cb + 1) * 128],
                                attn_t[:, 0, off:off + sz],
                                start=True, stop=False)
                            for m in range(1, MS):
                                nc.tensor.matmul(
                                    ops[:, :sz], vt[:, m, cb * 128:(cb + 1) * 128],
                                    attn_t[:, m, off:off + sz],
                                    start=False, stop=(m == MS - 1))
                            nc.vector.scalar_tensor_tensor(
                                o_sb[:, off:off + sz], ops[:, :sz], 0.0,
                                bc[:, off:off + sz],
                                op0=mybir.AluOpType.add,
                                op1=mybir.AluOpType.mult)
                        nc.gpsimd.dma_start(
                            out=out.ap()[cb * 128:(cb + 1) * 128,
                                         qt * QT:(qt + 1) * QT],
                            in_=o_sb)

                for qt in range(NQT):
                    st_phase(qt)
                    if qt >= 1:
                        av_phase(qt - 1)
                av_phase(NQT - 1)
    return nc


def _round_f32r(x):
    """Round-to-nearest-even to 11 mantissa bits (float32r semantics)."""
    u = np.ascontiguousarray(x, dtype=np.float32).view(np.uint32)
    rb = np.uint32(1 << 11)
    mask = np.uint32(0xFFFFF000)
    return ((u + rb) & mask).view(np.float32)


def kernel(left_features, right_features, wq, bq, wr, br):
    global LAST_RESULT
    left = np.asarray(left_features, dtype=np.float32)
    right = np.asarray(right_features, dtype=np.float32)
    wq = np.asarray(wq, dtype=np.float32)
    wr = np.asarray(wr, dtype=np.float32)
    bq = np.asarray(bq, dtype=np.float32)
    br = np.asarray(br, dtype=np.float32)

    lf = left.reshape(B, C, HW)
    rg = right.reshape(B, C, HW)
    lf_r = _round_f32r(lf)
    rg_r = _round_f32r(rg)
    wqt = _round_f32r(np.ascontiguousarray(wq.T))      # [C, D]
    wrt = _round_f32r(np.ascontiguousarray(wr.T))
    bq_t = np.ascontiguousarray(bq.reshape(DB, 128).T)  # [128, DB]
    br_t = np.ascontiguousarray(br.reshape(DB, 128).T)

    nc = build_kernel()
    in_maps = []
    for core in range(NCORES):
        b, d = core // 2, core % 2
        qf_c = lf_r[b] if d == 0 else rg_r[b]
        rf_c = rg_r[b] if d == 0 else lf_r[b]
        in_maps.append({
            "qf": np.ascontiguousarray(qf_c),
            "rf": np.ascontiguousarray(rf_c),
            "vtb": np.ascontiguousarray(rf_c.T.astype(ml_dtypes.bfloat16)),
            "wqt": wqt, "wrt": wrt, "bq": bq_t, "br": br_t,
        })
    res = run_bass_kernel_spmd(nc, in_maps, core_ids=list(range(NCORES)),
                               trace=TRACE)
    LAST_RESULT = res

    weighted = np.stack([res.results[core]["out"] for core in range(NCORES)])
    weighted = weighted.reshape(B, 2, C, 48, 48)
    left_att = np.concatenate([left, weighted[:, 0]], axis=1)
    right_att = np.concatenate([right, weighted[:, 1]], axis=1)
    return (left_att, right_att)
